# revision 3
# baseline (speedup 1.0000x reference)
"""Trainium2 Bass kernel: 2-layer LSTM over word embeddings + dense head.

Model (per reference):
  x = emb[tokens]                      # [B=64, S=512, E=300]
  h1 = LSTM_256(x); h2 = LSTM_256(h1)  # gates f,i,c(g),o ; combined z @ W
  out = sigmoid(relu(h2[:, -1] @ Wd + bd) @ Wout + bout)   # [B, 1]

Sharding: data-parallel over batch, 8 cores x 8 rows each; weights and the
embedding table replicated.

v2 layout (vs v1): input projections are accumulated in PSUM chunk tiles and
the recurrent matmuls accumulate directly on top, so gate nonlinearities read
the summed preactivation straight from PSUM.  This removes the per-step
identity-matmul PSUM injections and the per-chunk PSUM->SBUF CAST copies of
v1.  Per chunk of CH=8 steps and per layer:
  - bank A (double-buffered): f,i,o preacts, col = j*64 + tl*8 + b (j 0..5)
  - bank G (single-buffered): g preacts (j 6,7); g xpre is emitted at the
    chunk boundary after the last tanh(g) read, so one bank suffices.
Sigmoid(f,i,o) is ONE 48-col activation reading a [128,6,8] strided PSUM AP
(Scalar engine count per step-layer drops 4 -> 3); tanh(g) still streams
early, hidden under the f/i/o matmul tiles.  Input-projection weights are
fp8-e4m3 like the recurrent ones (fast-weight-load), and the L1 bias rides a
constant 1.0 column of the padded embedding table (row 300 of w1x chunk k=2),
so L1 xpre needs no rank-1 bias matmuls.  PSUM: 2 layers x (2xA + G) = 6
banks + 2 transpose/head banks = 8.
"""

import numpy as np
import ml_dtypes

BF16 = ml_dtypes.bfloat16
F8 = ml_dtypes.float8_e4m3

# Problem constants (hardcoded; kernel.py must be self-contained).
V, E, E_PAD = 50000, 300, 384
U = 256          # hidden units per LSTM layer
G4 = 4 * U       # 4 gates stacked: f, i, o, g
DNS = 128        # dense units
B, S = 64, 512
NCORES = 8
BL = B // NCORES  # batch rows per core = 8
CH = 8            # steps per xpre chunk

_BUILD_CACHE = {}


def _build(S_, reps=1):
    """Build the Bass program (shared SPMD across all cores)."""
    import concourse.bass as bass
    import concourse.bacc as bacc
    import concourse.mybir as mybir
    from concourse.tile import TileContext
    from concourse.bass import ts

    AF = mybir.ActivationFunctionType
    dt = mybir.dt
    f32, bf16, i32 = dt.float32, dt.bfloat16, dt.int32
    f8 = dt.float8e4

    T = S_ * BL            # tokens per core
    NCH = S_ // CH         # number of step chunks
    assert S_ % CH == 0 and T % 128 == 0

    nc = bacc.Bacc("TRN2", target_bir_lowering=False)

    # ---- DRAM I/O ----
    emb_d = nc.dram_tensor("emb", [V, E_PAD], bf16, kind="ExternalInput")
    tok_d = nc.dram_tensor("tok", [T, 1], i32, kind="ExternalInput")
    w1x_d = nc.dram_tensor("w1x", [128, 3 * G4], f8, kind="ExternalInput")
    w1h_d = nc.dram_tensor("w1h", [128, 2 * G4], f8, kind="ExternalInput")
    w2x_d = nc.dram_tensor("w2x", [128, 2 * G4], f8, kind="ExternalInput")
    w2h_d = nc.dram_tensor("w2h", [128, 2 * G4], f8, kind="ExternalInput")
    b2_d = nc.dram_tensor("b2", [1, G4], bf16, kind="ExternalInput")
    wd_d = nc.dram_tensor("wd", [128, 2 * DNS], bf16, kind="ExternalInput")
    bd_d = nc.dram_tensor("bd", [1, DNS], bf16, kind="ExternalInput")
    wo_d = nc.dram_tensor("wo", [128, 1], bf16, kind="ExternalInput")
    bo_d = nc.dram_tensor("bo", [1, 1], bf16, kind="ExternalInput")
    identb_d = nc.dram_tensor("identb", [128, 128], bf16, kind="ExternalInput")
    out_d = nc.dram_tensor("out", [1, BL], f32, kind="ExternalOutput")

    CW = CH * BL  # chunk width in psum cols = 64

    with TileContext(nc) as tc:
        from contextlib import ExitStack

        with ExitStack() as ex:
            stat = ex.enter_context(tc.tile_pool(name="static", bufs=1))
            tokp = ex.enter_context(tc.tile_pool(name="tokp", bufs=1))
            gthp = ex.enter_context(tc.tile_pool(name="gthp", bufs=1))
            actp = ex.enter_context(tc.tile_pool(name="actp", bufs=4))
            tmpp = ex.enter_context(tc.tile_pool(name="tmpp", bufs=8))
            pA1 = ex.enter_context(tc.tile_pool(name="pA1", bufs=2, space="PSUM"))
            pG1 = ex.enter_context(tc.tile_pool(name="pG1", bufs=1, space="PSUM"))
            pA2 = ex.enter_context(tc.tile_pool(name="pA2", bufs=2, space="PSUM"))
            pG2 = ex.enter_context(tc.tile_pool(name="pG2", bufs=1, space="PSUM"))
            psx = ex.enter_context(tc.tile_pool(name="psx", bufs=2, space="PSUM"))

            # ---- static SBUF tensors ----
            w1x = stat.tile([128, 3 * G4], f8, name="w1x_sb")
            w1h = stat.tile([128, 2 * G4], f8, name="w1h_sb")
            w2x = stat.tile([128, 2 * G4], f8, name="w2x_sb")
            w2h = stat.tile([128, 2 * G4], f8, name="w2h_sb")
            b2 = stat.tile([1, G4], bf16, name="b2_sb")
            ones = stat.tile([1, 512], bf16, name="ones_sb")
            wd = stat.tile([128, 2 * DNS], bf16, name="wd_sb")
            bd = stat.tile([1, DNS], bf16, name="bd_sb")
            wo = stat.tile([128, 1], bf16, name="wo_sb")
            bo = stat.tile([1, 1], bf16, name="bo_sb")
            identb = stat.tile([128, 128], bf16, name="identb_sb")
            xt = [stat.tile([128, T], bf16, name=f"xt{k}_sb") for k in range(3)]
            H1 = stat.tile([128, 16 * S_], bf16, name="h1_sb")
            H2 = stat.tile([128, 16 * S_], bf16, name="h2_sb")
            c1 = stat.tile([128, 32], f32, name="c1_sb")
            c2 = stat.tile([128, 32], f32, name="c2_sb")
            zh = stat.tile([128, 16], bf16, name="zh_sb")
            dns = stat.tile([128, BL], bf16, name="dns_sb")
            osb = stat.tile([1, BL], f32, name="o_sb")

            # ---- load weights / constants ----
            for sb_t, dr_t in [
                (w1x, w1x_d), (w1h, w1h_d), (w2x, w2x_d), (w2h, w2h_d),
                (b2, b2_d), (wd, wd_d), (bd, bd_d),
                (wo, wo_d), (bo, bo_d), (identb, identb_d),
            ]:
                nc.sync.dma_start(sb_t[:], dr_t[:])
            # repeated `reps` times for differential wall-clock timing
            for _rep in range(reps):
                nc.gpsimd.memset(ones[:], 1.0)
                nc.gpsimd.memset(c1[:], 0.0)
                nc.gpsimd.memset(c2[:], 0.0)
                nc.gpsimd.memset(zh[:], 0.0)

                # ---- embedding gather (token-major) + transpose to
                # feature-major xt[k][f, token], f = k*128 + p, token = t*8+b.
                nt = T // 128
                tka = tokp.tile([128, nt], i32, name="tka")
                nc.sync.dma_start(
                    tka[:].rearrange("p (i x) -> p i x", x=1),
                    tok_d[:].rearrange("(i p) x -> p i x", p=128))
                gall = gthp.tile([128, nt * E_PAD], bf16, name="gall")
                for i in range(nt):
                    nc.gpsimd.indirect_dma_start(
                        out=gall[:, i * E_PAD:(i + 1) * E_PAD],
                        out_offset=None,
                        in_=emb_d[:],
                        in_offset=bass.IndirectOffsetOnAxis(ap=tka[:, i:i + 1], axis=0),
                    )
                    for k in range(3):
                        pst = psx.tile([128, 128], bf16, name="pst", tag="psx")
                        nc.tensor.transpose(
                            pst[:],
                            gall[:, i * E_PAD + k * 128: i * E_PAD + (k + 1) * 128],
                            identb[:],
                        )
                        nc.vector.tensor_copy(xt[k][:, ts(i, 128)], pst[:])

                H1r = H1[:].rearrange("p (t r) -> p t r", r=16)

                # ---- chunked input projections, accumulated in PSUM ----
                def xpre1(c):
                    """L1 chunk c: returns (psA, psG) tiles holding the x-part
                    (+bias, via the constant emb column) for steps c*CH..+CH."""
                    psA = pA1.tile([128, 6 * CW], f32, name="a1", tag="a1")
                    psG = pG1.tile([128, 2 * CW], f32, name="g1", tag="g1")
                    for j in range(8):
                        ps, col = (psA, j * CW) if j < 6 else (psG, (j - 6) * CW)
                        for k in range(3):
                            nc.tensor.matmul(
                                ps[:, col:col + CW],
                                lhsT=w1x[:, k * G4 + j * 128: k * G4 + (j + 1) * 128],
                                rhs=xt[k][:, c * CW:(c + 1) * CW],
                                start=(k == 0), stop=False,
                                skip_group_check=True,
                            )
                    return psA, psG

                def xpre2_A(c, half, psA):
                    """L2 chunk c bank-A xpre for steps [c*CH + half*4, +4)."""
                    HW = CW // 2
                    t0 = c * CH + half * 4
                    for j in range(6):
                        col = j * CW + half * HW
                        nc.tensor.matmul(
                            psA[:, col:col + HW],
                            lhsT=b2[0:1, j * 128:(j + 1) * 128],
                            rhs=ones[0:1, 0:HW],
                            start=True, stop=False, skip_group_check=True,
                        )
                        for k in range(2):
                            nc.tensor.matmul(
                                psA[:, col:col + HW],
                                lhsT=w2x[:, k * G4 + j * 128: k * G4 + (j + 1) * 128],
                                rhs=H1r[:, t0:t0 + 4, k * 8:(k + 1) * 8],
                                start=False, stop=False, skip_group_check=True,
                            )

                def xpre2_G(c):
                    psG = pG2.tile([128, 2 * CW], f32, name="g2", tag="g2")
                    t0 = c * CH
                    for j in (6, 7):
                        col = (j - 6) * CW
                        nc.tensor.matmul(
                            psG[:, col:col + CW],
                            lhsT=b2[0:1, j * 128:(j + 1) * 128],
                            rhs=ones[0:1, 0:CW],
                            start=True, stop=False, skip_group_check=True,
                        )
                        for k in range(2):
                            nc.tensor.matmul(
                                psG[:, col:col + CW],
                                lhsT=w2x[:, k * G4 + j * 128: k * G4 + (j + 1) * 128],
                                rhs=H1r[:, t0:t0 + CH, k * 8:(k + 1) * 8],
                                start=False, stop=False, skip_group_check=True,
                            )
                    return psG

                # ---- one LSTM step: PE part (rec matmuls + gate ACTs) and
                # tail part (cell update), split so the two layers' engine
                # programs interleave as [L1 PE][L2 PE][L1 tail][L2 tail].
                def step_gates(psA, psG, wh, H, c_sb, acts, t, tl):
                    def hprev(k):
                        if t == 0:
                            return zh[:, k * 8:(k + 1) * 8]
                        return H[:, (t - 1) * 16 + k * 8:(t - 1) * 16 + (k + 1) * 8]

                    def rec_mm(ps, col, j):
                        for k in range(2):
                            nc.tensor.matmul(
                                ps[:, col + tl * 8: col + (tl + 1) * 8],
                                lhsT=wh[:, k * G4 + j * 128: k * G4 + (j + 1) * 128],
                                rhs=hprev(k),
                                start=False, stop=(k == 1), skip_group_check=True,
                            )

                    for j in (6, 7):                 # g first (own bank)
                        rec_mm(psG, (j - 6) * CW, j)
                    # tanh(g) streams while f/i/o tiles run on the PE
                    gview = psG[:].rearrange("p (j r) -> p j r", j=2)
                    nc.scalar.activation(
                        c_sb[:, 16:32], gview[:, :, tl * 8:(tl + 1) * 8], AF.Tanh)
                    for j in range(6):               # f, i, o
                        rec_mm(psA, j * CW, j)
                    aview = psA[:].rearrange("p (j r) -> p j r", j=6)
                    nc.scalar.activation(
                        acts[:], aview[:, :, tl * 8:(tl + 1) * 8], AF.Sigmoid)

                def step_tail(H, c_sb, acts, t):
                    # c_new = f*c + i*tanh(g); h = o * tanh(c_new)
                    pr = tmpp.tile([128, 32], f32, name="pr")
                    nc.vector.tensor_mul(pr[:], acts[:, 0:32], c_sb[:])
                    nc.vector.tensor_add(c_sb[:, 0:16], pr[:, 0:16], pr[:, 16:32])
                    th = tmpp.tile([128, 16], f32, name="th")
                    nc.scalar.activation(th[:], c_sb[:, 0:16], AF.Tanh)
                    nc.vector.tensor_mul(H[:, t * 16:(t + 1) * 16], acts[:, 32:48], th[:])

                # ---- main pipeline: L1 chunk c runs with L2 chunk c-1 ----
                a1A, a1G = xpre1(0)
                a2A = a2G = None
                p1A = p1G = p2A = p2G = None
                for c in range(NCH):
                    p1A, p1G = a1A, a1G
                    for tl in range(CH):
                        t = c * CH + tl
                        acts1 = actp.tile([128, 48], f32, name="acts1")
                        step_gates(p1A, p1G, w1h, H1, c1, acts1, t, tl)
                        if c >= 1:
                            acts2 = actp.tile([128, 48], f32, name="acts2")
                            step_gates(p2A, p2G, w2h, H2, c2, acts2, t - CH, tl)
                        step_tail(H1, c1, acts1, t)
                        if c >= 1:
                            step_tail(H2, c2, acts2, t - CH)
                        if tl == 3:
                            if c == 0:
                                a2A = pA2.tile([128, 6 * CW], f32, name="a2", tag="a2")
                            xpre2_A(c, 0, a2A)
                        if tl == CH - 1:
                            if c + 1 < NCH:
                                a1A, a1G = xpre1(c + 1)
                            a2G_new = xpre2_G(c)
                            xpre2_A(c, 1, a2A)
                            p2A, p2G = a2A, a2G_new
                            if c + 1 < NCH:
                                a2A = pA2.tile([128, 6 * CW], f32, name="a2", tag="a2")
                for tl in range(CH):  # layer-2 tail chunk
                    t = S_ - CH + tl
                    acts2 = actp.tile([128, 48], f32, name="acts2")
                    step_gates(p2A, p2G, w2h, H2, c2, acts2, t, tl)
                    step_tail(H2, c2, acts2, t)

                # ---- dense head on final h2 ----
                psd = psx.tile([128, 32], f32, name="psd", tag="psx")
                for k in range(2):
                    nc.tensor.matmul(
                        psd[:, 0:BL],
                        lhsT=wd[:, k * DNS:(k + 1) * DNS],
                        rhs=H2[:, (S_ - 1) * 16 + k * 8:(S_ - 1) * 16 + (k + 1) * 8],
                        start=(k == 0), stop=False,
                    )
                nc.tensor.matmul(psd[:, 0:BL], lhsT=bd[0:1, :], rhs=ones[0:1, 0:BL],
                                 start=False, stop=True, skip_group_check=True)
                nc.scalar.activation(dns[:], psd[:, 0:BL], AF.Relu)
                pso = psx.tile([128, 32], f32, name="pso", tag="psx")
                nc.tensor.matmul(pso[0:1, 0:BL], lhsT=wo[:, 0:1], rhs=dns[:],
                                 start=True, stop=False, skip_group_check=True)
                nc.tensor.matmul(pso[0:1, 0:BL], lhsT=bo[0:1, 0:1], rhs=ones[0:1, 0:BL],
                                 start=False, stop=True, skip_group_check=True)
                nc.scalar.activation(osb[:], pso[0:1, 0:BL], AF.Sigmoid)
                nc.sync.dma_start(out_d[:], osb[:])

    nc.compile()
    return nc


def _pack_weights(inputs):
    """Host-side packing into the device layouts (gate order f, i, o, g)."""
    f32 = np.float32

    def gates(prefix):
        return [np.asarray(inputs[prefix + g], f32) for g in ("f", "i", "o", "c")]

    W1 = gates("W1")   # each [E+U, U]
    W2 = gates("W2")   # each [2U, U]
    b1 = np.concatenate([np.asarray(inputs["b1" + g], f32) for g in ("f", "i", "o", "c")])
    b2 = np.concatenate([np.asarray(inputs["b2" + g], f32) for g in ("f", "i", "o", "c")])

    w1x_full = np.concatenate([w[:E] for w in W1], axis=1)        # [300, 1024]
    w1x_full = np.concatenate(
        [w1x_full, np.zeros((E_PAD - E, G4), f32)], axis=0)       # [384, 1024]
    w1x_full[E] = b1            # rides the constant 1.0 embedding column
    w1x = np.concatenate([w1x_full[k * 128:(k + 1) * 128] for k in range(3)],
                         axis=1).astype(F8)                       # [128, 3072]
    w1h_full = np.concatenate([w[E:] for w in W1], axis=1)        # [256, 1024]
    w1h = np.concatenate([w1h_full[k * 128:(k + 1) * 128] for k in range(2)],
                         axis=1).astype(F8)                       # [128, 2048]
    w2x_full = np.concatenate([w[:U] for w in W2], axis=1)
    w2x = np.concatenate([w2x_full[k * 128:(k + 1) * 128] for k in range(2)],
                         axis=1).astype(F8)
    w2h_full = np.concatenate([w[U:] for w in W2], axis=1)
    w2h = np.concatenate([w2h_full[k * 128:(k + 1) * 128] for k in range(2)],
                         axis=1).astype(F8)

    wd_full = np.asarray(inputs["Wd"], f32)                       # [256, 128]
    wd = np.concatenate([wd_full[k * 128:(k + 1) * 128] for k in range(2)],
                        axis=1).astype(BF16)                      # [128, 256]
    pack = {
        "w1x": w1x, "w1h": w1h, "w2x": w2x, "w2h": w2h,
        "b2": b2.astype(BF16).reshape(1, G4),
        "wd": wd,
        "bd": np.asarray(inputs["bd"], f32).astype(BF16).reshape(1, DNS),
        "wo": np.asarray(inputs["Wout"], f32).astype(BF16).reshape(128, 1),
        "bo": np.asarray(inputs["bout"], f32).astype(BF16).reshape(1, 1),
        "identb": np.eye(128, dtype=BF16),
    }
    emb = np.asarray(inputs["emb"], f32)
    emb_pad = np.zeros((V, E_PAD), BF16)
    emb_pad[:, :E] = emb.astype(BF16)
    emb_pad[:, E] = np.ones((), BF16)   # constant column carrying the L1 bias
    pack["emb"] = emb_pad
    return pack


def kernel(**inputs):
    from concourse.bass_utils import run_bass_kernel_spmd

    tokens = np.asarray(inputs["tokens"])
    S_ = tokens.shape[1]
    key = (S_,)
    if key not in _BUILD_CACHE:
        _BUILD_CACHE[key] = _build(S_)
    nc = _BUILD_CACHE[key]

    pack = _pack_weights(inputs)
    in_maps = []
    for core in range(NCORES):
        tok = tokens[core * BL:(core + 1) * BL].astype(np.int32)  # [8, S]
        tok = np.ascontiguousarray(tok.T).reshape(-1, 1)          # f = t*8 + b
        in_maps.append({**pack, "tok": tok})

    res = run_bass_kernel_spmd(nc, in_maps, core_ids=list(range(NCORES)))
    global _LAST_RESULTS
    _LAST_RESULTS = res
    out = np.concatenate(
        [r["out"].reshape(BL, 1) for r in res.results], axis=0
    ).astype(np.float32)
    return out


_LAST_RESULTS = None


# revision 9
# speedup vs baseline: 6.1367x; 6.1367x over previous
"""Trainium2 Bass kernel: 2-layer LSTM over word embeddings + dense head.

Model (per reference):
  x = emb[tokens]                      # [B=64, S=512, E=300]
  h1 = LSTM_256(x); h2 = LSTM_256(h1)  # gates f,i,c(g),o ; combined z @ W
  out = sigmoid(relu(h2[:, -1] @ Wd + bd) @ Wout + bout)   # [B, 1]

Sharding: data-parallel over batch, 8 cores x 8 rows each; weights and the
embedding table replicated.

v2 layout (vs v1): input projections are accumulated in PSUM chunk tiles and
the recurrent matmuls accumulate directly on top, so gate nonlinearities read
the summed preactivation straight from PSUM.  This removes the per-step
identity-matmul PSUM injections and the per-chunk PSUM->SBUF CAST copies of
v1.  Per chunk of CH=8 steps and per layer:
  - bank A (double-buffered): f,i,o preacts, col = j*64 + tl*8 + b (j 0..5)
  - bank G (single-buffered): g preacts (j 6,7); g xpre is emitted at the
    chunk boundary after the last tanh(g) read, so one bank suffices.
Sigmoid(f,i,o) is ONE 48-col activation reading a [128,6,8] strided PSUM AP
(Scalar engine count per step-layer drops 4 -> 3); tanh(g) still streams
early, hidden under the f/i/o matmul tiles.  Input-projection weights are
fp8-e4m3 like the recurrent ones (fast-weight-load), and the L1 bias rides a
constant 1.0 column of the padded embedding table (row 300 of w1x chunk k=2),
so L1 xpre needs no rank-1 bias matmuls.  PSUM: 2 layers x (2xA + G) = 6
banks + 2 transpose/head banks = 8.
"""

import numpy as np
import ml_dtypes

BF16 = ml_dtypes.bfloat16
F8 = ml_dtypes.float8_e4m3

# Problem constants (hardcoded; kernel.py must be self-contained).
V, E, E_PAD = 50000, 300, 384
U = 256          # hidden units per LSTM layer
G4 = 4 * U       # 4 gates stacked: f, i, o, g
DNS = 128        # dense units
B, S = 64, 512
NCORES = 8
BL = B // NCORES  # batch rows per core = 8
CH = 8            # steps per xpre chunk

_BUILD_CACHE = {}


def _build(S_, reps=1):
    """Build the Bass program (shared SPMD across all cores)."""
    import concourse.bass as bass
    import concourse.bacc as bacc
    import concourse.mybir as mybir
    from concourse.tile import TileContext
    from concourse.bass import ts

    AF = mybir.ActivationFunctionType
    dt = mybir.dt
    f32, bf16, i32 = dt.float32, dt.bfloat16, dt.int32
    f8 = dt.float8e4

    T = S_ * BL            # tokens per core
    NCH = S_ // CH         # number of step chunks
    assert S_ % CH == 0 and T % 128 == 0

    nc = bacc.Bacc("TRN2", target_bir_lowering=False)

    # ---- DRAM I/O ----
    emb_d = nc.dram_tensor("emb", [V, E_PAD], bf16, kind="ExternalInput")
    tok_d = nc.dram_tensor("tok", [T, 1], i32, kind="ExternalInput")
    w1x_d = nc.dram_tensor("w1x", [128, 3 * G4], f8, kind="ExternalInput")
    w1h_d = nc.dram_tensor("w1h", [128, 2 * G4], f8, kind="ExternalInput")
    w2x_d = nc.dram_tensor("w2x", [128, 2 * G4], f8, kind="ExternalInput")
    w2h_d = nc.dram_tensor("w2h", [128, 2 * G4], f8, kind="ExternalInput")
    b2_d = nc.dram_tensor("b2", [1, G4], bf16, kind="ExternalInput")
    wd_d = nc.dram_tensor("wd", [128, 2 * DNS], bf16, kind="ExternalInput")
    bd_d = nc.dram_tensor("bd", [1, DNS], bf16, kind="ExternalInput")
    wo_d = nc.dram_tensor("wo", [128, 1], bf16, kind="ExternalInput")
    bo_d = nc.dram_tensor("bo", [1, 1], bf16, kind="ExternalInput")
    identb_d = nc.dram_tensor("identb", [128, 128], bf16, kind="ExternalInput")
    out_d = nc.dram_tensor("out", [1, BL], f32, kind="ExternalOutput")

    CW = CH * BL  # chunk width in psum cols = 64

    with TileContext(nc) as tc:
        from contextlib import ExitStack

        with ExitStack() as ex:
            stat = ex.enter_context(tc.tile_pool(name="static", bufs=1))
            tokp = ex.enter_context(tc.tile_pool(name="tokp", bufs=1))
            gthp = ex.enter_context(tc.tile_pool(name="gthp", bufs=1))
            actp = ex.enter_context(tc.tile_pool(name="actp", bufs=4))
            tmpp = ex.enter_context(tc.tile_pool(name="tmpp", bufs=8))
            pA1 = ex.enter_context(tc.tile_pool(name="pA1", bufs=2, space="PSUM"))
            pG1 = ex.enter_context(tc.tile_pool(name="pG1", bufs=1, space="PSUM"))
            pA2 = ex.enter_context(tc.tile_pool(name="pA2", bufs=2, space="PSUM"))
            pG2 = ex.enter_context(tc.tile_pool(name="pG2", bufs=1, space="PSUM"))
            psx = ex.enter_context(tc.tile_pool(name="psx", bufs=2, space="PSUM"))

            # ---- static SBUF tensors ----
            w1x = stat.tile([128, 3 * G4], f8, name="w1x_sb")
            w1h = stat.tile([128, 2 * G4], f8, name="w1h_sb")
            w2x = stat.tile([128, 2 * G4], f8, name="w2x_sb")
            w2h = stat.tile([128, 2 * G4], f8, name="w2h_sb")
            b2 = stat.tile([1, G4], bf16, name="b2_sb")
            ones = stat.tile([1, 512], bf16, name="ones_sb")
            wd = stat.tile([128, 2 * DNS], bf16, name="wd_sb")
            bd = stat.tile([1, DNS], bf16, name="bd_sb")
            wo = stat.tile([128, 1], bf16, name="wo_sb")
            bo = stat.tile([1, 1], bf16, name="bo_sb")
            identb = stat.tile([128, 128], bf16, name="identb_sb")
            xt = [stat.tile([128, T], bf16, name=f"xt{k}_sb") for k in range(3)]
            H1 = stat.tile([128, 16 * S_], bf16, name="h1_sb")
            H2 = stat.tile([128, 16 * S_], bf16, name="h2_sb")
            c1 = stat.tile([128, 32], f32, name="c1_sb")
            c2 = stat.tile([128, 32], f32, name="c2_sb")
            zh = stat.tile([128, 16], bf16, name="zh_sb")
            dns = stat.tile([128, BL], bf16, name="dns_sb")
            osb = stat.tile([1, BL], f32, name="o_sb")

            # ---- load weights / constants ----
            for sb_t, dr_t in [
                (w1x, w1x_d), (w1h, w1h_d), (w2x, w2x_d), (w2h, w2h_d),
                (b2, b2_d), (wd, wd_d), (bd, bd_d),
                (wo, wo_d), (bo, bo_d), (identb, identb_d),
            ]:
                nc.sync.dma_start(sb_t[:], dr_t[:])
            # repeated `reps` times for differential wall-clock timing
            for _rep in range(reps):
                nc.gpsimd.memset(ones[:], 1.0)
                nc.gpsimd.memset(c1[:], 0.0)
                nc.gpsimd.memset(c2[:], 0.0)
                nc.gpsimd.memset(zh[:], 0.0)

                # ---- embedding gather (token-major) + transpose to
                # feature-major xt[k][f, token], f = k*128 + p, token = t*8+b.
                nt = T // 128
                tka = tokp.tile([128, nt], i32, name="tka")
                nc.sync.dma_start(
                    tka[:].rearrange("p (i x) -> p i x", x=1),
                    tok_d[:].rearrange("(i p) x -> p i x", p=128))
                gall = gthp.tile([128, nt * E_PAD], bf16, name="gall")
                for i in range(nt):
                    nc.gpsimd.indirect_dma_start(
                        out=gall[:, i * E_PAD:(i + 1) * E_PAD],
                        out_offset=None,
                        in_=emb_d[:],
                        in_offset=bass.IndirectOffsetOnAxis(ap=tka[:, i:i + 1], axis=0),
                    )
                    for k in range(3):
                        pst = psx.tile([128, 128], bf16, name="pst", tag="psx")
                        nc.tensor.transpose(
                            pst[:],
                            gall[:, i * E_PAD + k * 128: i * E_PAD + (k + 1) * 128],
                            identb[:],
                        )
                        nc.vector.tensor_copy(xt[k][:, ts(i, 128)], pst[:])

                H1r = H1[:].rearrange("p (t r) -> p t r", r=16)

                # ---- chunked input projections, accumulated in PSUM.
                # Emission is spread across the steps of the running chunk so
                # the in-order PE fills its chain-wait gaps with xpre tiles
                # instead of hitting a boundary lump.
                def xpre1_A(c, psA, jlo):
                    """L1 chunk c bank-A xpre, gate-chunks jlo, jlo+1."""
                    for j in (jlo, jlo + 1):
                        for k in range(3):
                            nc.tensor.matmul(
                                psA[:, j * CW:(j + 1) * CW],
                                lhsT=w1x[:, k * G4 + j * 128: k * G4 + (j + 1) * 128],
                                rhs=xt[k][:, c * CW:(c + 1) * CW],
                                start=(k == 0), stop=False,
                                skip_group_check=True,
                            )

                def xpre1_G(c):
                    psG = pG1.tile([128, 2 * CW], f32, name="g1", tag="g1")
                    for j in (6, 7):
                        col = (j - 6) * CW
                        for k in range(3):
                            nc.tensor.matmul(
                                psG[:, col:col + CW],
                                lhsT=w1x[:, k * G4 + j * 128: k * G4 + (j + 1) * 128],
                                rhs=xt[k][:, c * CW:(c + 1) * CW],
                                start=(k == 0), stop=False,
                                skip_group_check=True,
                            )
                    return psG

                def xpre2_A(c, psA, half, jlo):
                    """L2 chunk c bank-A xpre for steps [c*CH + half*4, +4),
                    gate-chunks jlo, jlo+1."""
                    HW = CW // 2
                    t0 = c * CH + half * 4
                    for j in (jlo, jlo + 1):
                        col = j * CW + half * HW
                        nc.tensor.matmul(
                            psA[:, col:col + HW],
                            lhsT=b2[0:1, j * 128:(j + 1) * 128],
                            rhs=ones[0:1, 0:HW],
                            start=True, stop=False, skip_group_check=True,
                        )
                        for k in range(2):
                            nc.tensor.matmul(
                                psA[:, col:col + HW],
                                lhsT=w2x[:, k * G4 + j * 128: k * G4 + (j + 1) * 128],
                                rhs=H1r[:, t0:t0 + 4, k * 8:(k + 1) * 8],
                                start=False, stop=False, skip_group_check=True,
                            )

                def xpre2_G(c):
                    psG = pG2.tile([128, 2 * CW], f32, name="g2", tag="g2")
                    t0 = c * CH
                    for j in (6, 7):
                        col = (j - 6) * CW
                        nc.tensor.matmul(
                            psG[:, col:col + CW],
                            lhsT=b2[0:1, j * 128:(j + 1) * 128],
                            rhs=ones[0:1, 0:CW],
                            start=True, stop=False, skip_group_check=True,
                        )
                        for k in range(2):
                            nc.tensor.matmul(
                                psG[:, col:col + CW],
                                lhsT=w2x[:, k * G4 + j * 128: k * G4 + (j + 1) * 128],
                                rhs=H1r[:, t0:t0 + CH, k * 8:(k + 1) * 8],
                                start=False, stop=False, skip_group_check=True,
                            )
                    return psG

                # ---- one LSTM step: PE part (rec matmuls + gate ACTs) and
                # tail part (cell update), split so the two layers' engine
                # programs interleave as [L1 PE][L2 PE][L1 tail][L2 tail].
                def step_gates(psA, psG, wh, H, c_sb, acts, t, tl):
                    def hprev(k):
                        if t == 0:
                            return zh[:, k * 8:(k + 1) * 8]
                        return H[:, (t - 1) * 16 + k * 8:(t - 1) * 16 + (k + 1) * 8]

                    def rec_mm(ps, col, j):
                        for k in range(2):
                            nc.tensor.matmul(
                                ps[:, col + tl * 8: col + (tl + 1) * 8],
                                lhsT=wh[:, k * G4 + j * 128: k * G4 + (j + 1) * 128],
                                rhs=hprev(k),
                                start=False, stop=(k == 1), skip_group_check=True,
                            )

                    for j in (6, 7):                 # g first (own bank)
                        rec_mm(psG, (j - 6) * CW, j)
                    # tanh(g) streams while f/i/o tiles run on the PE
                    gview = psG[:].rearrange("p (j r) -> p j r", j=2)
                    nc.scalar.activation(
                        c_sb[:, 16:32], gview[:, :, tl * 8:(tl + 1) * 8], AF.Tanh)
                    for j in range(6):               # f, i, o
                        rec_mm(psA, j * CW, j)
                    aview = psA[:].rearrange("p (j r) -> p j r", j=6)
                    nc.scalar.activation(
                        acts[:], aview[:, :, tl * 8:(tl + 1) * 8], AF.Sigmoid)

                def step_tail(H, c_sb, acts, t):
                    # c_new = f*c + i*tanh(g); h = o * tanh(c_new)
                    pr = tmpp.tile([128, 32], f32, name="pr")
                    nc.vector.tensor_mul(pr[:], acts[:, 0:32], c_sb[:])
                    nc.vector.tensor_add(c_sb[:, 0:16], pr[:, 0:16], pr[:, 16:32])
                    th = tmpp.tile([128, 16], f32, name="th")
                    nc.scalar.activation(th[:], c_sb[:, 0:16], AF.Tanh)
                    nc.vector.tensor_mul(H[:, t * 16:(t + 1) * 16], acts[:, 32:48], th[:])

                # ---- main pipeline: L1 chunk c runs with L2 chunk c-1 ----
                l1A_cur = pA1.tile([128, 6 * CW], f32, name="a1", tag="a1")
                for jlo in (0, 2, 4):
                    xpre1_A(0, l1A_cur, jlo)
                l1G_cur = xpre1_G(0)
                l1A_nxt = l1G_nxt = None
                l2A_cur = l2G_cur = None   # L2 chunk c-1 tiles
                l2A_nxt = l2G_nxt = None   # L2 chunk c tiles
                for c in range(NCH):
                    for tl in range(CH):
                        t = c * CH + tl
                        acts1 = actp.tile([128, 48], f32, name="acts1")
                        step_gates(l1A_cur, l1G_cur, w1h, H1, c1, acts1, t, tl)
                        if c >= 1:
                            acts2 = actp.tile([128, 48], f32, name="acts2")
                            step_gates(l2A_cur, l2G_cur, w2h, H2, c2, acts2,
                                       t - CH, tl)
                        step_tail(H1, c1, acts1, t)
                        if c >= 1:
                            step_tail(H2, c2, acts2, t - CH)
                        # spread xpre emission into this step's PE gap
                        if tl in (0, 1, 2):
                            if c + 1 < NCH:
                                if tl == 0:
                                    l1A_nxt = pA1.tile([128, 6 * CW], f32,
                                                       name="a1", tag="a1")
                                xpre1_A(c + 1, l1A_nxt, 2 * tl)
                            if c >= 1:   # L2 chunk c-1, second half
                                xpre2_A(c - 1, l2A_cur, 1, 2 * tl)
                        elif tl in (4, 5, 6):
                            if tl == 4:
                                l2A_nxt = pA2.tile([128, 6 * CW], f32,
                                                   name="a2", tag="a2")
                            xpre2_A(c, l2A_nxt, 0, 2 * (tl - 4))
                        elif tl == CH - 1:  # boundary (g banks single-buffered)
                            if c + 1 < NCH:
                                l1G_nxt = xpre1_G(c + 1)
                            l2G_nxt = xpre2_G(c)
                    l1A_cur, l1G_cur = l1A_nxt, l1G_nxt
                    l2A_cur, l2G_cur = l2A_nxt, l2G_nxt
                for tl in range(CH):  # layer-2 tail chunk
                    t = S_ - CH + tl
                    acts2 = actp.tile([128, 48], f32, name="acts2")
                    step_gates(l2A_cur, l2G_cur, w2h, H2, c2, acts2, t, tl)
                    step_tail(H2, c2, acts2, t)
                    if tl in (0, 1, 2):   # last chunk's second half
                        xpre2_A(NCH - 1, l2A_cur, 1, 2 * tl)

                # ---- dense head on final h2 ----
                psd = psx.tile([128, 32], f32, name="psd", tag="psx")
                for k in range(2):
                    nc.tensor.matmul(
                        psd[:, 0:BL],
                        lhsT=wd[:, k * DNS:(k + 1) * DNS],
                        rhs=H2[:, (S_ - 1) * 16 + k * 8:(S_ - 1) * 16 + (k + 1) * 8],
                        start=(k == 0), stop=False,
                    )
                nc.tensor.matmul(psd[:, 0:BL], lhsT=bd[0:1, :], rhs=ones[0:1, 0:BL],
                                 start=False, stop=True, skip_group_check=True)
                nc.scalar.activation(dns[:], psd[:, 0:BL], AF.Relu)
                pso = psx.tile([128, 32], f32, name="pso", tag="psx")
                nc.tensor.matmul(pso[0:1, 0:BL], lhsT=wo[:, 0:1], rhs=dns[:],
                                 start=True, stop=False, skip_group_check=True)
                nc.tensor.matmul(pso[0:1, 0:BL], lhsT=bo[0:1, 0:1], rhs=ones[0:1, 0:BL],
                                 start=False, stop=True, skip_group_check=True)
                nc.scalar.activation(osb[:], pso[0:1, 0:BL], AF.Sigmoid)
                nc.sync.dma_start(out_d[:], osb[:])

    nc.compile()
    return nc


def _pack_weights(inputs):
    """Host-side packing into the device layouts (gate order f, i, o, g)."""
    f32 = np.float32

    def gates(prefix):
        return [np.asarray(inputs[prefix + g], f32) for g in ("f", "i", "o", "c")]

    W1 = gates("W1")   # each [E+U, U]
    W2 = gates("W2")   # each [2U, U]
    b1 = np.concatenate([np.asarray(inputs["b1" + g], f32) for g in ("f", "i", "o", "c")])
    b2 = np.concatenate([np.asarray(inputs["b2" + g], f32) for g in ("f", "i", "o", "c")])

    w1x_full = np.concatenate([w[:E] for w in W1], axis=1)        # [300, 1024]
    w1x_full = np.concatenate(
        [w1x_full, np.zeros((E_PAD - E, G4), f32)], axis=0)       # [384, 1024]
    w1x_full[E] = b1            # rides the constant 1.0 embedding column
    w1x = np.concatenate([w1x_full[k * 128:(k + 1) * 128] for k in range(3)],
                         axis=1).astype(F8)                       # [128, 3072]
    w1h_full = np.concatenate([w[E:] for w in W1], axis=1)        # [256, 1024]
    w1h = np.concatenate([w1h_full[k * 128:(k + 1) * 128] for k in range(2)],
                         axis=1).astype(F8)                       # [128, 2048]
    w2x_full = np.concatenate([w[:U] for w in W2], axis=1)
    w2x = np.concatenate([w2x_full[k * 128:(k + 1) * 128] for k in range(2)],
                         axis=1).astype(F8)
    w2h_full = np.concatenate([w[U:] for w in W2], axis=1)
    w2h = np.concatenate([w2h_full[k * 128:(k + 1) * 128] for k in range(2)],
                         axis=1).astype(F8)

    wd_full = np.asarray(inputs["Wd"], f32)                       # [256, 128]
    wd = np.concatenate([wd_full[k * 128:(k + 1) * 128] for k in range(2)],
                        axis=1).astype(BF16)                      # [128, 256]
    pack = {
        "w1x": w1x, "w1h": w1h, "w2x": w2x, "w2h": w2h,
        "b2": b2.astype(BF16).reshape(1, G4),
        "wd": wd,
        "bd": np.asarray(inputs["bd"], f32).astype(BF16).reshape(1, DNS),
        "wo": np.asarray(inputs["Wout"], f32).astype(BF16).reshape(128, 1),
        "bo": np.asarray(inputs["bout"], f32).astype(BF16).reshape(1, 1),
        "identb": np.eye(128, dtype=BF16),
    }
    emb = np.asarray(inputs["emb"], f32)
    emb_pad = np.zeros((V, E_PAD), BF16)
    emb_pad[:, :E] = emb.astype(BF16)
    emb_pad[:, E] = np.ones((), BF16)   # constant column carrying the L1 bias
    pack["emb"] = emb_pad
    return pack


def kernel(**inputs):
    from concourse.bass_utils import run_bass_kernel_spmd

    tokens = np.asarray(inputs["tokens"])
    S_ = tokens.shape[1]
    key = (S_,)
    if key not in _BUILD_CACHE:
        _BUILD_CACHE[key] = _build(S_)
    nc = _BUILD_CACHE[key]

    pack = _pack_weights(inputs)
    in_maps = []
    for core in range(NCORES):
        tok = tokens[core * BL:(core + 1) * BL].astype(np.int32)  # [8, S]
        tok = np.ascontiguousarray(tok.T).reshape(-1, 1)          # f = t*8 + b
        in_maps.append({**pack, "tok": tok})

    res = run_bass_kernel_spmd(nc, in_maps, core_ids=list(range(NCORES)))
    global _LAST_RESULTS
    _LAST_RESULTS = res
    out = np.concatenate(
        [r["out"].reshape(BL, 1) for r in res.results], axis=0
    ).astype(np.float32)
    return out


_LAST_RESULTS = None
